# revision 17
# baseline (speedup 1.0000x reference)
"""CTC loss kernel for Trainium2 (8 NeuronCores, data-parallel over batch).

Algorithm: CTC forward DP as 33 "layer scans" (one DVE tensor_tensor_scan
per extended-target position s), in a scaled linear domain
E = exp(logit - ln2).  Per layer:
    A[t, s] = (A[t-1, s-1] + m[s] * A[t-1, s-2] + A[t-1, s]) * E[t, s]
The per-(b,t) softmax normalizer sum_c exp(logit_c - ln2) is computed
separately and stitched on the host:  ll = log(A_final) - sum_t log(sum37).

v2 layout/engine plan (from HW microbenchmarks):
  - All big HBM traffic in bf16 (halves DMA bytes vs f32).
  - Scan feed pre-gathered AND pre-transposed on host to
    [class, group, p, t] so each scan's d1 operand is contiguous; DMA'd in
    3 class-chunks so the layer-0 scan starts ~3-4us into the kernel.
  - DVE: only the serial scan+STT chain (~56us busy) plus tiny finals;
    this is the span driver (scan = 2.38 ns/col, dtype-insensitive).
  - sum37: ACT exp (bf16->bf16) then a pairwise bf16 tensor_tensor
    add-tree on gpsimd (Pool) — replaces DVE tensor_reduce (20us) with
    ~16us of otherwise-idle Pool time, fully overlapped with the chain.
  - Phase 3 has no dependency on the chain, so it overlaps completely.
"""

import math
from contextlib import ExitStack

import numpy as np

B, T, C, L = 2048, 256, 37, 16
BLANK = 36
S = 2 * L + 1               # 33 layers
NCORES = 8
BC = B // NCORES            # 256 samples per core
G = 2                       # sample groups of 128 partitions per core
P = 128
CE = L + 1                  # classes in the scan feed (blank first, then 16 labels)
TS = T + 1                  # per-group slots incl. 1 pad slot
NSCAN = G * T + 1           # scan length: g0 t0..255, pad, g1 t0..255
GBIAS = -1.0 * math.log(2.0)   # exp bias: E = exp(x - ln2)
NCHUNK = 4                  # t-chunks for the logi/e37 path
TC = T // NCHUNK

GPS_TREE = False            # Pool TT is slow AND contends with DVE: keep tree on DVE

_cache = {}


def _build(gps_tree):
    import concourse.bass as bass
    import concourse.bacc as bacc
    import concourse.mybir as mybir
    import concourse.tile as tile

    f32 = mybir.dt.float32
    bf16 = mybir.dt.bfloat16
    AF = mybir.ActivationFunctionType
    ALU = mybir.AluOpType
    AX = mybir.AxisListType

    nc = bacc.Bacc("TRN2", target_bir_lowering=False, debug=False)

    # pre-kernel constants (ACT bias APs must exist before the Tile program:
    # ACT instructions may wait on only one semaphore)
    _gb = nc.alloc_sbuf_tensor("const-float32-gbias", [128, 1], f32)
    nc.gpsimd.memset(_gb.ap(), GBIAS)
    nc.const_aps.aps[(f32, GBIAS)] = _gb.ap()
    _z0c = nc.alloc_sbuf_tensor("const-float32-zero", [128, 1], f32)
    nc.gpsimd.memset(_z0c.ap(), 0.0)
    nc.const_aps.aps[(f32, 0.0)] = _z0c.ap()
    nc.all_engine_barrier()

    # scan feed: host-gathered label columns of logits, [class, group, p, t]
    # (class order: blank, l0..l15), bf16
    sft = nc.dram_tensor("sft", [CE, G, P, T], bf16, kind="ExternalInput")
    logi = nc.dram_tensor("logi", [BC, T, C], bf16, kind="ExternalInput")
    msk = nc.dram_tensor("msk", [P, L], f32, kind="ExternalInput")
    outv = nc.dram_tensor("outv", [BC, 3], f32, kind="ExternalOutput")

    # class-chunks in layer consumption order; blank-only first chunk so the
    # layer-0 scan starts as early as possible
    CCHUNKS = [(0, 1), (1, 2), (2, 9), (9, 17)]

    with tile.TileContext(nc) as tc, ExitStack() as ctx:
        pool1 = ctx.enter_context(tc.tile_pool(name="res", bufs=1))

        # --- resident tiles ---
        stg = pool1.tile([P, CE * G * T], bf16, tag="stg")        # bf16 staging
        e17 = pool1.tile([P, CE * G * TS], f32, tag="e17")        # exp'd feed (+pads)
        e37 = pool1.tile([P, G * T * C], bf16, tag="e37")         # exp'd logits
        mtile = pool1.tile([P, L], f32, tag="mtile")              # per-partition masks
        z0 = pool1.tile([P, NSCAN], f32, tag="z0")
        l0 = pool1.tile([P, NSCAN + 1], f32, tag="l0")
        lbufs = [pool1.tile([P, NSCAN + 1], f32, tag=f"lb{i}", name=f"lb{i}")
                 for i in range(4)]
        vts = [pool1.tile([P, NSCAN], f32, tag=f"vt{i}", name=f"vt{i}")
               for i in range(2)]
        # add-tree intermediates (bf16)
        tr18 = pool1.tile([P, G * T * 18], bf16, tag="tr18")
        tr9 = pool1.tile([P, G * T * 9], bf16, tag="tr9")
        tr4 = pool1.tile([P, G * T * 4], bf16, tag="tr4")
        tr2 = pool1.tile([P, G * T * 2], bf16, tag="tr2")
        tr1 = pool1.tile([P, G * T], bf16, tag="tr1")
        tlo = pool1.tile([P, G * T], bf16, tag="tlo")
        sum37 = pool1.tile([P, G * T], f32, tag="sum37")
        lg37 = pool1.tile([P, G * T], f32, tag="lg37")
        lsum2 = pool1.tile([P, G], f32, tag="lsum2")
        outt = pool1.tile([P, G * 3], f32, tag="outt")

        e17v = e17[:].rearrange("p (c g t) -> p c g t", c=CE, g=G, t=TS)
        e17f = e17[:].rearrange("p (c n) -> p c n", c=CE)    # n = G*TS = 514
        stgv = stg[:].rearrange("p (c g t) -> p c g t", c=CE, g=G, t=T)
        sfv = sft.ap().rearrange("c g p t -> p c g t")

        # --- phase 1: scan feed, class-chunked DMA + exp (bf16 -> f32) ---
        # pad slots (t=256 of each group) zeroed once; exp only writes t<256.
        # small init memsets on gpsimd to keep them off the DVE/ACT streams
        nc.gpsimd.memset(e17v[:, :, :, T], 0.0)
        # masks first: tiny transfer, must not queue behind the big chunks
        # (host pre-laid-out [P, L]: contiguous per partition)
        nc.sync.dma_start(mtile[:], msk.ap())
        for c0, c1 in CCHUNKS:
            dst = stgv[:, c0:c1, :, :]
            nc.sync.dma_start(dst, sfv[:, c0:c1, :, :])
            nc.scalar.activation(e17v[:, c0:c1, :, 0:T], dst, AF.Exp, bias=GBIAS)
        nc.gpsimd.memset(z0[:], 0.0)
        nc.gpsimd.memset(z0[:, T + 1:T + 2], 1.0)   # A_g1[-1,0] injection
        nc.gpsimd.memset(l0[:, 0:1], 1.0)           # A_g0[-1,0] = 1
        for lb in lbufs:
            nc.gpsimd.memset(lb[:, 0:1], 0.0)       # A_g0[-1,s>=1] = 0

        # --- phase 3 DMA+exp emitted early (ACT runs them as DMAs land) ---
        lgv = logi.ap().rearrange("(g p) t c -> p g t c", g=G, p=P)
        e37v = e37[:].rearrange("p (g t c) -> p g t c", g=G, t=T, c=C)
        for ci in range(NCHUNK):
            tsl = slice(ci * TC, (ci + 1) * TC)
            dst = e37v[:, :, tsl, :]
            nc.sync.dma_start(dst, lgv[:, :, tsl, :])
            nc.scalar.activation(dst, dst, AF.Exp, bias=GBIAS)

        # --- phase 2: the 33 layer scans (concatenated groups, 513 cols) ---
        def lbuf(s):
            return l0 if s == 0 else lbufs[(s - 1) % 4]

        # host pairs samples so both groups of a partition share one mask
        # vector (mismatched pairs are recomputed host-side): ONE 513-col STT
        # per odd layer instead of two 257-col ones.
        for s in range(S):
            dst = lbuf(s)
            ci = 0 if s % 2 == 0 else (s - 1) // 2 + 1
            d1 = e17f[:, ci, 0:NSCAN]
            if s == 0:
                nc.vector.tensor_tensor_scan(
                    dst[:, 1:NSCAN + 1], z0[:], d1, 1.0, ALU.add, ALU.mult)
                # overwrite pad output: A_g1[-1, 0] = 1 for layer 1's d0 read
                nc.vector.memset(dst[:, T + 1:T + 2], 1.0)
                continue
            if s == 1 or s % 2 == 0:
                d0 = lbuf(s - 1)[:, 0:NSCAN]
            else:
                k = (s - 1) // 2
                v = vts[k % 2]
                nc.vector.scalar_tensor_tensor(
                    v[:, 0:NSCAN], lbuf(s - 2)[:, 0:NSCAN],
                    mtile[:, k:k + 1], lbuf(s - 1)[:, 0:NSCAN],
                    ALU.mult, ALU.add)
                d0 = v[:, 0:NSCAN]
            nc.vector.tensor_tensor_scan(
                dst[:, 1:NSCAN + 1], d0, d1, 0.0, ALU.add, ALU.mult)

        # --- phase 3 tree + ln: one-shot full-T bf16 add-tree on DVE ---
        TRE = nc.gpsimd if gps_tree else nc.vector
        t18 = tr18[:].rearrange("p (g t c) -> p g t c", g=G, t=T, c=18)
        t9 = tr9[:].rearrange("p (g t c) -> p g t c", g=G, t=T, c=9)
        t4 = tr4[:].rearrange("p (g t c) -> p g t c", g=G, t=T, c=4)
        t2 = tr2[:].rearrange("p (g t c) -> p g t c", g=G, t=T, c=2)
        t1 = tr1[:].rearrange("p (g t) -> p g t", g=G)
        tl = tlo[:].rearrange("p (g t) -> p g t", g=G)
        s37 = sum37[:].rearrange("p (g t) -> p g t", g=G)
        TRE.tensor_tensor(t18[:, :, :, 0:18], e37v[:, :, :, 0:18],
                          e37v[:, :, :, 18:36], ALU.add)
        TRE.tensor_tensor(t9[:, :, :, 0:9], t18[:, :, :, 0:9],
                          t18[:, :, :, 9:18], ALU.add)
        TRE.tensor_tensor(t4[:, :, :, 0:4], t9[:, :, :, 0:4],
                          t9[:, :, :, 4:8], ALU.add)
        TRE.tensor_tensor(tl[:, :, :], t9[:, :, :, 8], e37v[:, :, :, 36], ALU.add)
        TRE.tensor_tensor(t2[:, :, :, 0:2], t4[:, :, :, 0:2],
                          t4[:, :, :, 2:4], ALU.add)
        TRE.tensor_tensor(t1[:, :, :], t2[:, :, :, 0], t2[:, :, :, 1], ALU.add)
        TRE.tensor_tensor(s37[:, :, :], t1[:, :, :], tl[:, :, :], ALU.add)
        nc.scalar.activation(lg37[:], sum37[:], AF.Ln, bias=0.0)
        lg = lg37[:].rearrange("p (g t) -> p g t", g=G)

        # --- phase 4: finals on DVE (after the chain) + output ---
        nc.vector.tensor_reduce(lsum2[:], lg, AX.X, ALU.add)
        ot = outt[:].rearrange("p (g i) -> p g i", g=G)
        l31, l32 = lbuf(S - 2), lbuf(S - 1)
        # g0 final at buffer col T (scan col 255); g1 final at col NSCAN
        nc.vector.tensor_copy(ot[:, 0, 0:1], l31[:, T:T + 1])
        nc.vector.tensor_copy(ot[:, 1, 0:1], l31[:, NSCAN:NSCAN + 1])
        nc.vector.tensor_copy(ot[:, 0, 1:2], l32[:, T:T + 1])
        nc.vector.tensor_copy(ot[:, 1, 1:2], l32[:, NSCAN:NSCAN + 1])
        nc.vector.tensor_copy(ot[:, :, 2], lsum2[:])
        nc.sync.dma_start(
            outv.ap().rearrange("(g p) i -> p g i", g=G, p=P), outt[:],
        )

    nc.compile()
    return nc


def _pairing(mask_core):
    """Arrange 256 samples into (g0, g1) partition pairs with identical mask
    vectors where possible. Returns perm (new order: first 128 = g0, last
    128 = g1) and the list of partitions whose pair is mismatched."""
    keys = {}
    for i in range(BC):
        keys.setdefault(mask_core[i].tobytes(), []).append(i)
    pairs, single = [], []
    for lst in keys.values():
        while len(lst) >= 2:
            pairs.append((lst.pop(), lst.pop()))
        if lst:
            single.append(lst.pop())
    bad = []
    while single:
        a = single.pop()
        b = single.pop() if single else a   # can't happen: BC even & parity
        bad.append(len(pairs))
        pairs.append((a, b))
    g0 = np.array([p[0] for p in pairs], np.int64)
    g1 = np.array([p[1] for p in pairs], np.int64)
    return np.concatenate([g0, g1]), bad


def _host_prep(logits, targets):
    import ml_dtypes
    tgt = targets.reshape(B, L)
    mask = np.zeros((B, L), np.float32)
    mask[:, 1:] = (tgt[:, 1:] != tgt[:, :-1]).astype(np.float32)
    # per-core pairing permutation (identical mask vectors share a partition)
    perms = np.empty((NCORES, BC), np.int64)
    bad_global = []          # global sample ids whose device mask was wrong
    for ci in range(NCORES):
        pm, bad = _pairing(mask[ci * BC:(ci + 1) * BC])
        perms[ci] = pm + ci * BC
        for pi in bad:
            bad_global.append(int(perms[ci][P + pi]))   # the g1 sample
    order = perms.reshape(-1)
    logits_p = logits[order]
    tgt_p = tgt[order]
    mask_p = mask[order]
    # gathered label columns + blank -> [B, T, 17], blank first
    sf = np.empty((B, T, CE), np.float32)
    sf[:, :, 0] = logits_p[:, :, BLANK]
    sf[:, :, 1:] = np.take_along_axis(
        logits_p, np.broadcast_to(tgt_p[:, None, :], (B, T, L)), axis=2
    )
    # -> [core, class, group, p, t]
    sfT = np.ascontiguousarray(
        sf.reshape(NCORES, G, P, T, CE).transpose(0, 4, 1, 2, 3)
    ).astype(ml_dtypes.bfloat16)
    logi16 = logits_p.astype(ml_dtypes.bfloat16)
    # per-partition mask = the g0 sample's mask, [core, P, L]
    maskT = np.ascontiguousarray(
        mask_p.reshape(NCORES, G, P, L)[:, 0])
    return sfT, logi16, maskT, order, bad_global


def _ll_exact(logits, tgt):
    """Per-sample CTC log-likelihood, f64 log-domain (for overrides)."""
    n = logits.shape[0]
    x = logits.astype(np.float64)
    m = x.max(-1, keepdims=True)
    lp = x - (np.log(np.exp(x - m).sum(-1, keepdims=True)) + m)
    ext = np.full((n, S), BLANK, np.int64)
    ext[:, 1::2] = tgt
    lpe = np.take_along_axis(lp, ext[:, None, :].repeat(T, 1), axis=2)
    skip = (ext != BLANK) & np.concatenate(
        [np.zeros((n, 2), bool), ext[:, 2:] != ext[:, :-2]], 1)
    NEG = -1e30

    def lae(a, b):
        mm = np.maximum(a, b)
        return mm + np.log1p(np.exp(-np.abs(a - b)))

    al = np.full((n, S), NEG)
    al[:, 0] = lpe[:, 0, 0]
    al[:, 1] = lpe[:, 0, 1]
    for t in range(1, T):
        a1 = np.concatenate([np.full((n, 1), NEG), al[:, :-1]], 1)
        a2 = np.where(skip, np.concatenate([np.full((n, 2), NEG), al[:, :-2]], 1), NEG)
        al = lae(lae(al, a1), a2) + lpe[:, t]
    return lae(al[:, S - 2], al[:, S - 1])


def kernel(logits, targets, input_lengths, target_lengths):
    logits = np.asarray(logits, np.float32)
    targets = np.asarray(targets, np.int32)
    assert logits.shape == (B, T, C)

    from concourse import bass_utils

    if "nc" not in _cache:
        _cache["nc"] = _build(GPS_TREE)
    nc = _cache["nc"]

    sfT, logi16, maskT, order, bad = _host_prep(logits, targets)
    in_maps = []
    for ci in range(NCORES):
        sl = slice(ci * BC, (ci + 1) * BC)
        in_maps.append({
            "sft": sfT[ci],
            "logi": np.ascontiguousarray(logi16[sl]),
            "msk": maskT[ci],
        })
    res = bass_utils.run_bass_kernel_spmd(nc, in_maps, core_ids=list(range(NCORES)))
    outs = np.concatenate([r["outv"] for r in res.results], axis=0)  # [B, 3] permuted
    a31 = outs[:, 0].astype(np.float64)
    a32 = outs[:, 1].astype(np.float64)
    lz = outs[:, 2].astype(np.float64)
    ll_p = np.log(a31 + a32) - lz
    ll = np.empty(B, np.float64)
    ll[order] = ll_p
    if bad:
        bi = np.array(bad, np.int64)
        ll[bi] = _ll_exact(logits[bi], targets.reshape(B, L)[bi])
    loss = np.mean(-ll / L)
    return np.float32(loss)


# revision 19
# speedup vs baseline: 1.0024x; 1.0024x over previous
"""CTC loss kernel for Trainium2 (8 NeuronCores, data-parallel over batch).

Algorithm: CTC forward DP as 33 "layer scans" (one DVE tensor_tensor_scan
per extended-target position s), in a scaled linear domain
E = exp(logit - ln2).  Per layer:
    A[t, s] = (A[t-1, s-1] + m[s] * A[t-1, s-2] + A[t-1, s]) * E[t, s]
The per-(b,t) softmax normalizer sum_c exp(logit_c - ln2) is computed
separately and stitched on the host:  ll = log(A_final) - sum_t log(sum37).

v2 layout/engine plan (from HW microbenchmarks):
  - All big HBM traffic in bf16 (halves DMA bytes vs f32).
  - Scan feed pre-gathered AND pre-transposed on host to
    [class, group, p, t] so each scan's d1 operand is contiguous; DMA'd in
    3 class-chunks so the layer-0 scan starts ~3-4us into the kernel.
  - DVE: only the serial scan+STT chain (~56us busy) plus tiny finals;
    this is the span driver (scan = 2.38 ns/col, dtype-insensitive).
  - sum37: ACT exp (bf16->bf16) then a pairwise bf16 tensor_tensor
    add-tree on gpsimd (Pool) — replaces DVE tensor_reduce (20us) with
    ~16us of otherwise-idle Pool time, fully overlapped with the chain.
  - Phase 3 has no dependency on the chain, so it overlaps completely.
"""

import math
from contextlib import ExitStack

import numpy as np

B, T, C, L = 2048, 256, 37, 16
BLANK = 36
S = 2 * L + 1               # 33 layers
NCORES = 8
BC = B // NCORES            # 256 samples per core
G = 2                       # sample groups of 128 partitions per core
P = 128
CE = L + 1                  # classes in the scan feed (blank first, then 16 labels)
TS = T + 1                  # per-group slots incl. 1 pad slot
NSCAN = G * T + 1           # scan length: g0 t0..255, pad, g1 t0..255
GBIAS = -1.0 * math.log(2.0)   # exp bias: E = exp(x - ln2)
NCHUNK = 4                  # t-chunks for the logi/e37 path
TC = T // NCHUNK

GPS_TREE = False            # Pool TT is slow AND contends with DVE: keep tree on DVE

_cache = {}


def _build(gps_tree):
    import concourse.bass as bass
    import concourse.bacc as bacc
    import concourse.mybir as mybir
    import concourse.tile as tile

    f32 = mybir.dt.float32
    bf16 = mybir.dt.bfloat16
    AF = mybir.ActivationFunctionType
    ALU = mybir.AluOpType
    AX = mybir.AxisListType

    nc = bacc.Bacc("TRN2", target_bir_lowering=False, debug=False)

    # pre-kernel constants (ACT bias APs must exist before the Tile program:
    # ACT instructions may wait on only one semaphore)
    _gb = nc.alloc_sbuf_tensor("const-float32-gbias", [128, 1], f32)
    nc.gpsimd.memset(_gb.ap(), GBIAS)
    nc.const_aps.aps[(f32, GBIAS)] = _gb.ap()
    _z0c = nc.alloc_sbuf_tensor("const-float32-zero", [128, 1], f32)
    nc.gpsimd.memset(_z0c.ap(), 0.0)
    nc.const_aps.aps[(f32, 0.0)] = _z0c.ap()
    nc.all_engine_barrier()

    # scan feed: host-gathered label columns of logits, [class, group, p, t]
    # (class order: blank, l0..l15), bf16
    sft = nc.dram_tensor("sft", [CE, G, P, T], bf16, kind="ExternalInput")
    logi = nc.dram_tensor("logi", [BC, T, C], bf16, kind="ExternalInput")
    msk = nc.dram_tensor("msk", [P, L], f32, kind="ExternalInput")
    outv = nc.dram_tensor("outv", [BC, 3], f32, kind="ExternalOutput")

    # class-chunks in layer consumption order; fine-grained so each layer's
    # E row is exp'd just ahead of its scan (layer 2k+1 consumes class k+1)
    CCHUNKS = [(0, 1), (1, 2), (2, 3), (3, 5), (5, 9), (9, 13), (13, 17)]

    with tile.TileContext(nc) as tc, ExitStack() as ctx:
        pool1 = ctx.enter_context(tc.tile_pool(name="res", bufs=1))

        # --- resident tiles ---
        stg = pool1.tile([P, CE * G * T], bf16, tag="stg")        # bf16 staging
        e17 = pool1.tile([P, CE * G * TS], f32, tag="e17")        # exp'd feed (+pads)
        e37 = pool1.tile([P, G * T * C], bf16, tag="e37")         # exp'd logits
        mtile = pool1.tile([P, L], f32, tag="mtile")              # per-partition masks
        z0 = pool1.tile([P, NSCAN], f32, tag="z0")
        l0 = pool1.tile([P, NSCAN + 1], f32, tag="l0")
        lbufs = [pool1.tile([P, NSCAN + 1], f32, tag=f"lb{i}", name=f"lb{i}")
                 for i in range(4)]
        vts = [pool1.tile([P, NSCAN], f32, tag=f"vt{i}", name=f"vt{i}")
               for i in range(2)]
        # add-tree intermediates (bf16)
        tr18 = pool1.tile([P, G * T * 18], bf16, tag="tr18")
        tr9 = pool1.tile([P, G * T * 9], bf16, tag="tr9")
        tr4 = pool1.tile([P, G * T * 4], bf16, tag="tr4")
        tr2 = pool1.tile([P, G * T * 2], bf16, tag="tr2")
        tr1 = pool1.tile([P, G * T], bf16, tag="tr1")
        tlo = pool1.tile([P, G * T], bf16, tag="tlo")
        sum37 = pool1.tile([P, G * T], f32, tag="sum37")
        lg37 = pool1.tile([P, G * T], f32, tag="lg37")
        lsum2 = pool1.tile([P, G], f32, tag="lsum2")
        outt = pool1.tile([P, G * 3], f32, tag="outt")

        e17v = e17[:].rearrange("p (c g t) -> p c g t", c=CE, g=G, t=TS)
        e17f = e17[:].rearrange("p (c n) -> p c n", c=CE)    # n = G*TS = 514
        stgv = stg[:].rearrange("p (c g t) -> p c g t", c=CE, g=G, t=T)
        sfv = sft.ap().rearrange("c g p t -> p c g t")

        # --- phase 1: scan feed, class-chunked DMA + exp (bf16 -> f32) ---
        # pad slots (t=256 of each group) zeroed once; exp only writes t<256.
        # small init memsets on gpsimd to keep them off the DVE/ACT streams
        nc.gpsimd.memset(e17v[:, :, :, T], 0.0)
        # masks first: tiny transfer, must not queue behind the big chunks
        # (host pre-laid-out [P, L]: contiguous per partition)
        nc.sync.dma_start(mtile[:], msk.ap())
        for c0, c1 in CCHUNKS:
            dst = stgv[:, c0:c1, :, :]
            nc.sync.dma_start(dst, sfv[:, c0:c1, :, :])
            nc.scalar.activation(e17v[:, c0:c1, :, 0:T], dst, AF.Exp, bias=GBIAS)
        nc.gpsimd.memset(z0[:], 0.0)
        nc.gpsimd.memset(z0[:, T + 1:T + 2], 1.0)   # A_g1[-1,0] injection
        nc.gpsimd.memset(l0[:, 0:1], 1.0)           # A_g0[-1,0] = 1
        for lb in lbufs:
            nc.gpsimd.memset(lb[:, 0:1], 0.0)       # A_g0[-1,s>=1] = 0

        # --- phase 3 DMA+exp emitted early (ACT runs them as DMAs land) ---
        lgv = logi.ap().rearrange("(g p) t c -> p g t c", g=G, p=P)
        e37v = e37[:].rearrange("p (g t c) -> p g t c", g=G, t=T, c=C)
        for ci in range(NCHUNK):
            tsl = slice(ci * TC, (ci + 1) * TC)
            dst = e37v[:, :, tsl, :]
            nc.sync.dma_start(dst, lgv[:, :, tsl, :])
            nc.scalar.activation(dst, dst, AF.Exp, bias=GBIAS)

        # --- phase 2: the 33 layer scans (concatenated groups, 513 cols) ---
        def lbuf(s):
            return l0 if s == 0 else lbufs[(s - 1) % 4]

        # host pairs samples so both groups of a partition share one mask
        # vector (mismatched pairs are recomputed host-side): ONE 513-col STT
        # per odd layer instead of two 257-col ones.
        for s in range(S):
            dst = lbuf(s)
            ci = 0 if s % 2 == 0 else (s - 1) // 2 + 1
            d1 = e17f[:, ci, 0:NSCAN]
            if s == 0:
                nc.vector.tensor_tensor_scan(
                    dst[:, 1:NSCAN + 1], z0[:], d1, 1.0, ALU.add, ALU.mult)
                # overwrite pad output: A_g1[-1, 0] = 1 for layer 1's d0 read
                nc.vector.memset(dst[:, T + 1:T + 2], 1.0)
                continue
            if s == 1 or s % 2 == 0:
                d0 = lbuf(s - 1)[:, 0:NSCAN]
            else:
                k = (s - 1) // 2
                v = vts[k % 2]
                nc.vector.scalar_tensor_tensor(
                    v[:, 0:NSCAN], lbuf(s - 2)[:, 0:NSCAN],
                    mtile[:, k:k + 1], lbuf(s - 1)[:, 0:NSCAN],
                    ALU.mult, ALU.add)
                d0 = v[:, 0:NSCAN]
            nc.vector.tensor_tensor_scan(
                dst[:, 1:NSCAN + 1], d0, d1, 0.0, ALU.add, ALU.mult)

        # --- phase 3 tree + ln: one-shot full-T bf16 add-tree on DVE ---
        TRE = nc.gpsimd if gps_tree else nc.vector
        t18 = tr18[:].rearrange("p (g t c) -> p g t c", g=G, t=T, c=18)
        t9 = tr9[:].rearrange("p (g t c) -> p g t c", g=G, t=T, c=9)
        t4 = tr4[:].rearrange("p (g t c) -> p g t c", g=G, t=T, c=4)
        t2 = tr2[:].rearrange("p (g t c) -> p g t c", g=G, t=T, c=2)
        t1 = tr1[:].rearrange("p (g t) -> p g t", g=G)
        tl = tlo[:].rearrange("p (g t) -> p g t", g=G)
        s37 = sum37[:].rearrange("p (g t) -> p g t", g=G)
        TRE.tensor_tensor(t18[:, :, :, 0:18], e37v[:, :, :, 0:18],
                          e37v[:, :, :, 18:36], ALU.add)
        TRE.tensor_tensor(t9[:, :, :, 0:9], t18[:, :, :, 0:9],
                          t18[:, :, :, 9:18], ALU.add)
        TRE.tensor_tensor(t4[:, :, :, 0:4], t9[:, :, :, 0:4],
                          t9[:, :, :, 4:8], ALU.add)
        TRE.tensor_tensor(tl[:, :, :], t9[:, :, :, 8], e37v[:, :, :, 36], ALU.add)
        TRE.tensor_tensor(t2[:, :, :, 0:2], t4[:, :, :, 0:2],
                          t4[:, :, :, 2:4], ALU.add)
        TRE.tensor_tensor(t1[:, :, :], t2[:, :, :, 0], t2[:, :, :, 1], ALU.add)
        TRE.tensor_tensor(s37[:, :, :], t1[:, :, :], tl[:, :, :], ALU.add)
        nc.scalar.activation(lg37[:], sum37[:], AF.Ln, bias=0.0)
        lg = lg37[:].rearrange("p (g t) -> p g t", g=G)

        # --- phase 4: finals on DVE (after the chain) + output ---
        nc.vector.tensor_reduce(lsum2[:], lg, AX.X, ALU.add)
        ot = outt[:].rearrange("p (g i) -> p g i", g=G)
        l31, l32 = lbuf(S - 2), lbuf(S - 1)
        # per-group finals at buffer cols {T, NSCAN} = {256, 513}: one strided
        # [P, 2] copy per layer (view with period T+1 picks col 256 of each)
        t31 = l31[:, 0:2 * (T + 1)].rearrange("p (g t) -> p g t", g=G)
        t32 = l32[:, 0:2 * (T + 1)].rearrange("p (g t) -> p g t", g=G)
        nc.vector.tensor_copy(ot[:, :, 0], t31[:, :, T])
        nc.vector.tensor_copy(ot[:, :, 1], t32[:, :, T])
        nc.vector.tensor_copy(ot[:, :, 2], lsum2[:])
        nc.sync.dma_start(
            outv.ap().rearrange("(g p) i -> p g i", g=G, p=P), outt[:],
        )

    nc.compile()
    return nc


def _pairing(mask_core):
    """Arrange 256 samples into (g0, g1) partition pairs with identical mask
    vectors where possible. Returns perm (new order: first 128 = g0, last
    128 = g1) and the list of partitions whose pair is mismatched."""
    keys = {}
    for i in range(BC):
        keys.setdefault(mask_core[i].tobytes(), []).append(i)
    pairs, single = [], []
    for lst in keys.values():
        while len(lst) >= 2:
            pairs.append((lst.pop(), lst.pop()))
        if lst:
            single.append(lst.pop())
    bad = []
    while single:
        a = single.pop()
        b = single.pop() if single else a   # can't happen: BC even & parity
        bad.append(len(pairs))
        pairs.append((a, b))
    g0 = np.array([p[0] for p in pairs], np.int64)
    g1 = np.array([p[1] for p in pairs], np.int64)
    return np.concatenate([g0, g1]), bad


def _host_prep(logits, targets):
    import ml_dtypes
    tgt = targets.reshape(B, L)
    mask = np.zeros((B, L), np.float32)
    mask[:, 1:] = (tgt[:, 1:] != tgt[:, :-1]).astype(np.float32)
    # per-core pairing permutation (identical mask vectors share a partition)
    perms = np.empty((NCORES, BC), np.int64)
    bad_global = []          # global sample ids whose device mask was wrong
    for ci in range(NCORES):
        pm, bad = _pairing(mask[ci * BC:(ci + 1) * BC])
        perms[ci] = pm + ci * BC
        for pi in bad:
            bad_global.append(int(perms[ci][P + pi]))   # the g1 sample
    order = perms.reshape(-1)
    logits_p = logits[order]
    tgt_p = tgt[order]
    mask_p = mask[order]
    # gathered label columns + blank -> [B, T, 17], blank first
    sf = np.empty((B, T, CE), np.float32)
    sf[:, :, 0] = logits_p[:, :, BLANK]
    sf[:, :, 1:] = np.take_along_axis(
        logits_p, np.broadcast_to(tgt_p[:, None, :], (B, T, L)), axis=2
    )
    # -> [core, class, group, p, t]
    sfT = np.ascontiguousarray(
        sf.reshape(NCORES, G, P, T, CE).transpose(0, 4, 1, 2, 3)
    ).astype(ml_dtypes.bfloat16)
    logi16 = logits_p.astype(ml_dtypes.bfloat16)
    # per-partition mask = the g0 sample's mask, [core, P, L]
    maskT = np.ascontiguousarray(
        mask_p.reshape(NCORES, G, P, L)[:, 0])
    return sfT, logi16, maskT, order, bad_global


def _ll_exact(logits, tgt):
    """Per-sample CTC log-likelihood, f64 log-domain (for overrides)."""
    n = logits.shape[0]
    x = logits.astype(np.float64)
    m = x.max(-1, keepdims=True)
    lp = x - (np.log(np.exp(x - m).sum(-1, keepdims=True)) + m)
    ext = np.full((n, S), BLANK, np.int64)
    ext[:, 1::2] = tgt
    lpe = np.take_along_axis(lp, ext[:, None, :].repeat(T, 1), axis=2)
    skip = (ext != BLANK) & np.concatenate(
        [np.zeros((n, 2), bool), ext[:, 2:] != ext[:, :-2]], 1)
    NEG = -1e30

    def lae(a, b):
        mm = np.maximum(a, b)
        return mm + np.log1p(np.exp(-np.abs(a - b)))

    al = np.full((n, S), NEG)
    al[:, 0] = lpe[:, 0, 0]
    al[:, 1] = lpe[:, 0, 1]
    for t in range(1, T):
        a1 = np.concatenate([np.full((n, 1), NEG), al[:, :-1]], 1)
        a2 = np.where(skip, np.concatenate([np.full((n, 2), NEG), al[:, :-2]], 1), NEG)
        al = lae(lae(al, a1), a2) + lpe[:, t]
    return lae(al[:, S - 2], al[:, S - 1])


def kernel(logits, targets, input_lengths, target_lengths):
    logits = np.asarray(logits, np.float32)
    targets = np.asarray(targets, np.int32)
    assert logits.shape == (B, T, C)

    from concourse import bass_utils

    if "nc" not in _cache:
        _cache["nc"] = _build(GPS_TREE)
    nc = _cache["nc"]

    sfT, logi16, maskT, order, bad = _host_prep(logits, targets)
    in_maps = []
    for ci in range(NCORES):
        sl = slice(ci * BC, (ci + 1) * BC)
        in_maps.append({
            "sft": sfT[ci],
            "logi": np.ascontiguousarray(logi16[sl]),
            "msk": maskT[ci],
        })
    res = bass_utils.run_bass_kernel_spmd(nc, in_maps, core_ids=list(range(NCORES)))
    outs = np.concatenate([r["outv"] for r in res.results], axis=0)  # [B, 3] permuted
    a31 = outs[:, 0].astype(np.float64)
    a32 = outs[:, 1].astype(np.float64)
    lz = outs[:, 2].astype(np.float64)
    ll_p = np.log(a31 + a32) - lz
    ll = np.empty(B, np.float64)
    ll[order] = ll_p
    if bad:
        bi = np.array(bad, np.int64)
        ll[bi] = _ll_exact(logits[bi], targets.reshape(B, L)[bi])
    loss = np.mean(-ll / L)
    return np.float32(loss)


# revision 21
# speedup vs baseline: 1.0497x; 1.0472x over previous
"""CTC loss kernel for Trainium2 (8 NeuronCores, data-parallel over batch).

Algorithm: CTC forward DP as 33 "layer scans" (one DVE tensor_tensor_scan
per extended-target position s), in a scaled linear domain
E = exp(logit - ln2).  Per layer:
    A[t, s] = (A[t-1, s-1] + m[s] * A[t-1, s-2] + A[t-1, s]) * E[t, s]
The per-(b,t) softmax normalizer sum_c exp(logit_c - ln2) is computed
separately and stitched on the host:  ll = log(A_final) - sum_t log(sum37).

v2 layout/engine plan (from HW microbenchmarks):
  - All big HBM traffic in bf16 (halves DMA bytes vs f32).
  - Scan feed pre-gathered AND pre-transposed on host to
    [class, group, p, t] so each scan's d1 operand is contiguous; DMA'd in
    3 class-chunks so the layer-0 scan starts ~3-4us into the kernel.
  - DVE: only the serial scan+STT chain (~56us busy) plus tiny finals;
    this is the span driver (scan = 2.38 ns/col, dtype-insensitive).
  - sum37: ACT exp (bf16->bf16) then a pairwise bf16 tensor_tensor
    add-tree on gpsimd (Pool) — replaces DVE tensor_reduce (20us) with
    ~16us of otherwise-idle Pool time, fully overlapped with the chain.
  - Phase 3 has no dependency on the chain, so it overlaps completely.
"""

import math
from contextlib import ExitStack

import numpy as np

B, T, C, L = 2048, 256, 37, 16
BLANK = 36
S = 2 * L + 1               # 33 layers
NCORES = 8
BC = B // NCORES            # 256 samples per core
G = 2                       # sample groups of 128 partitions per core
P = 128
CE = L + 1                  # classes in the scan feed (blank first, then 16 labels)
TS = T + 1                  # per-group slots incl. 1 pad slot
NSCAN = G * T + 1           # scan length: g0 t0..255, pad, g1 t0..255
GBIAS = -1.0 * math.log(2.0)   # exp bias: E = exp(x - ln2)
NCHUNK = 4                  # t-chunks for the logi/e37 path
TC = T // NCHUNK

GPS_TREE = False            # Pool TT is slow AND contends with DVE: keep tree on DVE

_cache = {}


def _build(gps_tree):
    import concourse.bass as bass
    import concourse.bacc as bacc
    import concourse.mybir as mybir
    import concourse.tile as tile

    f32 = mybir.dt.float32
    bf16 = mybir.dt.bfloat16
    AF = mybir.ActivationFunctionType
    ALU = mybir.AluOpType
    AX = mybir.AxisListType

    nc = bacc.Bacc("TRN2", target_bir_lowering=False, debug=False)

    # pre-kernel constants (ACT bias APs must exist before the Tile program:
    # ACT instructions may wait on only one semaphore)
    _gb = nc.alloc_sbuf_tensor("const-float32-gbias", [128, 1], f32)
    nc.gpsimd.memset(_gb.ap(), GBIAS)
    nc.const_aps.aps[(f32, GBIAS)] = _gb.ap()
    _z0c = nc.alloc_sbuf_tensor("const-float32-zero", [128, 1], f32)
    nc.gpsimd.memset(_z0c.ap(), 0.0)
    nc.const_aps.aps[(f32, 0.0)] = _z0c.ap()
    nc.all_engine_barrier()

    # scan feed: host-gathered label columns of logits, [class, group, p, t]
    # (class order: blank, l0..l15), bf16
    sft = nc.dram_tensor("sft", [CE, G, P, T], bf16, kind="ExternalInput")
    logi = nc.dram_tensor("logi", [BC, T, C], bf16, kind="ExternalInput")
    msk = nc.dram_tensor("msk", [P, L], f32, kind="ExternalInput")
    outv = nc.dram_tensor("outv", [BC, 3], f32, kind="ExternalOutput")

    # class-chunks in layer consumption order; single-class head chunks so
    # early layers are never feed-stalled (layer 2k+1 consumes class k+1)
    CCHUNKS = [(0, 1), (1, 2), (2, 3), (3, 9), (9, 17)]

    with tile.TileContext(nc) as tc, ExitStack() as ctx:
        pool1 = ctx.enter_context(tc.tile_pool(name="res", bufs=1))

        # --- resident tiles ---
        stg = pool1.tile([P, CE * G * T], bf16, tag="stg")        # bf16 staging
        e17 = pool1.tile([P, CE * G * TS], f32, tag="e17")        # exp'd feed (+pads)
        e37 = pool1.tile([P, G * T * C], bf16, tag="e37")         # exp'd logits
        mtile = pool1.tile([P, L], f32, tag="mtile")              # per-partition masks
        z0 = pool1.tile([P, NSCAN], f32, tag="z0")
        l0 = pool1.tile([P, NSCAN + 1], f32, tag="l0")
        lbufs = [pool1.tile([P, NSCAN + 1], f32, tag=f"lb{i}", name=f"lb{i}")
                 for i in range(4)]
        vts = [pool1.tile([P, NSCAN], f32, tag=f"vt{i}", name=f"vt{i}")
               for i in range(2)]
        # add-tree intermediates (bf16)
        tr18 = pool1.tile([P, G * T * 18], bf16, tag="tr18")
        tr9 = pool1.tile([P, G * T * 9], bf16, tag="tr9")
        tr4 = pool1.tile([P, G * T * 4], bf16, tag="tr4")
        tr2 = pool1.tile([P, G * T * 2], bf16, tag="tr2")
        tr1 = pool1.tile([P, G * T], bf16, tag="tr1")
        tlo = pool1.tile([P, G * T], bf16, tag="tlo")
        sum37 = pool1.tile([P, G * T], f32, tag="sum37")
        lg37 = pool1.tile([P, G * T], f32, tag="lg37")
        lsum2 = pool1.tile([P, G], f32, tag="lsum2")
        outt = pool1.tile([P, G * 3], f32, tag="outt")

        e17v = e17[:].rearrange("p (c g t) -> p c g t", c=CE, g=G, t=TS)
        e17f = e17[:].rearrange("p (c n) -> p c n", c=CE)    # n = G*TS = 514
        stgv = stg[:].rearrange("p (c g t) -> p c g t", c=CE, g=G, t=T)
        sfv = sft.ap().rearrange("c g p t -> p c g t")

        # --- phase 1: scan feed, class-chunked DMA + exp (bf16 -> f32) ---
        # pad slots (t=256 of each group) zeroed once; exp only writes t<256.
        # small init memsets on gpsimd to keep them off the DVE/ACT streams
        nc.gpsimd.memset(e17v[:, :, :, T], 0.0)
        for i, (c0, c1) in enumerate(CCHUNKS):
            dst = stgv[:, c0:c1, :, :]
            nc.sync.dma_start(dst, sfv[:, c0:c1, :, :])
            nc.scalar.activation(e17v[:, c0:c1, :, 0:T], dst, AF.Exp, bias=GBIAS)
            if i == 2:
                # masks: tiny transfer, after the head chunks (so the layer-0
                # feed isn't delayed) but before the fat tail chunks (so the
                # first STT isn't); host pre-laid-out [P, L]
                nc.sync.dma_start(mtile[:], msk.ap())
        nc.gpsimd.memset(z0[:], 0.0)
        nc.gpsimd.memset(z0[:, T + 1:T + 2], 1.0)   # A_g1[-1,0] injection
        nc.gpsimd.memset(l0[:, 0:1], 1.0)           # A_g0[-1,0] = 1
        for lb in lbufs:
            nc.gpsimd.memset(lb[:, 0:1], 0.0)       # A_g0[-1,s>=1] = 0

        # --- phase 3 DMA+exp emitted early (ACT runs them as DMAs land) ---
        lgv = logi.ap().rearrange("(g p) t c -> p g t c", g=G, p=P)
        e37v = e37[:].rearrange("p (g t c) -> p g t c", g=G, t=T, c=C)
        for ci in range(NCHUNK):
            tsl = slice(ci * TC, (ci + 1) * TC)
            dst = e37v[:, :, tsl, :]
            nc.sync.dma_start(dst, lgv[:, :, tsl, :])
            nc.scalar.activation(dst, dst, AF.Exp, bias=GBIAS)

        # --- phase 2: the 33 layer scans (concatenated groups, 513 cols) ---
        def lbuf(s):
            return l0 if s == 0 else lbufs[(s - 1) % 4]

        # host pairs samples so both groups of a partition share one mask
        # vector (mismatched pairs are recomputed host-side): ONE 513-col STT
        # per odd layer instead of two 257-col ones.
        for s in range(S):
            dst = lbuf(s)
            ci = 0 if s % 2 == 0 else (s - 1) // 2 + 1
            d1 = e17f[:, ci, 0:NSCAN]
            if s == 0:
                nc.vector.tensor_tensor_scan(
                    dst[:, 1:NSCAN + 1], z0[:], d1, 1.0, ALU.add, ALU.mult)
                # overwrite pad output: A_g1[-1, 0] = 1 for layer 1's d0 read
                nc.vector.memset(dst[:, T + 1:T + 2], 1.0)
                continue
            if s == 1 or s % 2 == 0:
                d0 = lbuf(s - 1)[:, 0:NSCAN]
            else:
                k = (s - 1) // 2
                v = vts[k % 2]
                nc.vector.scalar_tensor_tensor(
                    v[:, 0:NSCAN], lbuf(s - 2)[:, 0:NSCAN],
                    mtile[:, k:k + 1], lbuf(s - 1)[:, 0:NSCAN],
                    ALU.mult, ALU.add)
                d0 = v[:, 0:NSCAN]
            nc.vector.tensor_tensor_scan(
                dst[:, 1:NSCAN + 1], d0, d1, 0.0, ALU.add, ALU.mult)

        # --- phase 3 tree + ln: one-shot full-T bf16 add-tree on DVE ---
        TRE = nc.gpsimd if gps_tree else nc.vector
        t18 = tr18[:].rearrange("p (g t c) -> p g t c", g=G, t=T, c=18)
        t9 = tr9[:].rearrange("p (g t c) -> p g t c", g=G, t=T, c=9)
        t4 = tr4[:].rearrange("p (g t c) -> p g t c", g=G, t=T, c=4)
        t2 = tr2[:].rearrange("p (g t c) -> p g t c", g=G, t=T, c=2)
        t1 = tr1[:].rearrange("p (g t) -> p g t", g=G)
        tl = tlo[:].rearrange("p (g t) -> p g t", g=G)
        s37 = sum37[:].rearrange("p (g t) -> p g t", g=G)
        TRE.tensor_tensor(t18[:, :, :, 0:18], e37v[:, :, :, 0:18],
                          e37v[:, :, :, 18:36], ALU.add)
        TRE.tensor_tensor(t9[:, :, :, 0:9], t18[:, :, :, 0:9],
                          t18[:, :, :, 9:18], ALU.add)
        TRE.tensor_tensor(t4[:, :, :, 0:4], t9[:, :, :, 0:4],
                          t9[:, :, :, 4:8], ALU.add)
        TRE.tensor_tensor(tl[:, :, :], t9[:, :, :, 8], e37v[:, :, :, 36], ALU.add)
        TRE.tensor_tensor(t2[:, :, :, 0:2], t4[:, :, :, 0:2],
                          t4[:, :, :, 2:4], ALU.add)
        TRE.tensor_tensor(t1[:, :, :], t2[:, :, :, 0], t2[:, :, :, 1], ALU.add)
        TRE.tensor_tensor(s37[:, :, :], t1[:, :, :], tl[:, :, :], ALU.add)
        nc.scalar.activation(lg37[:], sum37[:], AF.Ln, bias=0.0)
        lg = lg37[:].rearrange("p (g t) -> p g t", g=G)

        # --- phase 4: finals on DVE (after the chain) + output ---
        nc.vector.tensor_reduce(lsum2[:], lg, AX.X, ALU.add)
        ot = outt[:].rearrange("p (g i) -> p g i", g=G)
        l31, l32 = lbuf(S - 2), lbuf(S - 1)
        # per-group finals at buffer cols {T, NSCAN} = {256, 513}: one strided
        # [P, 2] copy per layer (view with period T+1 picks col 256 of each)
        t31 = l31[:, 0:2 * (T + 1)].rearrange("p (g t) -> p g t", g=G)
        t32 = l32[:, 0:2 * (T + 1)].rearrange("p (g t) -> p g t", g=G)
        nc.vector.tensor_copy(ot[:, :, 0], t31[:, :, T])
        nc.vector.tensor_copy(ot[:, :, 1], t32[:, :, T])
        nc.vector.tensor_copy(ot[:, :, 2], lsum2[:])
        nc.sync.dma_start(
            outv.ap().rearrange("(g p) i -> p g i", g=G, p=P), outt[:],
        )

    nc.compile()
    return nc


def _pairing(mask_core):
    """Arrange 256 samples into (g0, g1) partition pairs with identical mask
    vectors where possible. Returns perm (new order: first 128 = g0, last
    128 = g1) and the list of partitions whose pair is mismatched."""
    keys = {}
    for i in range(BC):
        keys.setdefault(mask_core[i].tobytes(), []).append(i)
    pairs, single = [], []
    for lst in keys.values():
        while len(lst) >= 2:
            pairs.append((lst.pop(), lst.pop()))
        if lst:
            single.append(lst.pop())
    bad = []
    while single:
        a = single.pop()
        b = single.pop() if single else a   # can't happen: BC even & parity
        bad.append(len(pairs))
        pairs.append((a, b))
    g0 = np.array([p[0] for p in pairs], np.int64)
    g1 = np.array([p[1] for p in pairs], np.int64)
    return np.concatenate([g0, g1]), bad


def _host_prep(logits, targets):
    import ml_dtypes
    tgt = targets.reshape(B, L)
    mask = np.zeros((B, L), np.float32)
    mask[:, 1:] = (tgt[:, 1:] != tgt[:, :-1]).astype(np.float32)
    # per-core pairing permutation (identical mask vectors share a partition)
    perms = np.empty((NCORES, BC), np.int64)
    bad_global = []          # global sample ids whose device mask was wrong
    for ci in range(NCORES):
        pm, bad = _pairing(mask[ci * BC:(ci + 1) * BC])
        perms[ci] = pm + ci * BC
        for pi in bad:
            bad_global.append(int(perms[ci][P + pi]))   # the g1 sample
    order = perms.reshape(-1)
    logits_p = logits[order]
    tgt_p = tgt[order]
    mask_p = mask[order]
    # gathered label columns + blank -> [B, T, 17], blank first
    sf = np.empty((B, T, CE), np.float32)
    sf[:, :, 0] = logits_p[:, :, BLANK]
    sf[:, :, 1:] = np.take_along_axis(
        logits_p, np.broadcast_to(tgt_p[:, None, :], (B, T, L)), axis=2
    )
    # -> [core, class, group, p, t]
    sfT = np.ascontiguousarray(
        sf.reshape(NCORES, G, P, T, CE).transpose(0, 4, 1, 2, 3)
    ).astype(ml_dtypes.bfloat16)
    logi16 = logits_p.astype(ml_dtypes.bfloat16)
    # per-partition mask = the g0 sample's mask, [core, P, L]
    maskT = np.ascontiguousarray(
        mask_p.reshape(NCORES, G, P, L)[:, 0])
    return sfT, logi16, maskT, order, bad_global


def _ll_exact(logits, tgt):
    """Per-sample CTC log-likelihood, f64 log-domain (for overrides)."""
    n = logits.shape[0]
    x = logits.astype(np.float64)
    m = x.max(-1, keepdims=True)
    lp = x - (np.log(np.exp(x - m).sum(-1, keepdims=True)) + m)
    ext = np.full((n, S), BLANK, np.int64)
    ext[:, 1::2] = tgt
    lpe = np.take_along_axis(lp, ext[:, None, :].repeat(T, 1), axis=2)
    skip = (ext != BLANK) & np.concatenate(
        [np.zeros((n, 2), bool), ext[:, 2:] != ext[:, :-2]], 1)
    NEG = -1e30

    def lae(a, b):
        mm = np.maximum(a, b)
        return mm + np.log1p(np.exp(-np.abs(a - b)))

    al = np.full((n, S), NEG)
    al[:, 0] = lpe[:, 0, 0]
    al[:, 1] = lpe[:, 0, 1]
    for t in range(1, T):
        a1 = np.concatenate([np.full((n, 1), NEG), al[:, :-1]], 1)
        a2 = np.where(skip, np.concatenate([np.full((n, 2), NEG), al[:, :-2]], 1), NEG)
        al = lae(lae(al, a1), a2) + lpe[:, t]
    return lae(al[:, S - 2], al[:, S - 1])


def kernel(logits, targets, input_lengths, target_lengths):
    logits = np.asarray(logits, np.float32)
    targets = np.asarray(targets, np.int32)
    assert logits.shape == (B, T, C)

    from concourse import bass_utils

    if "nc" not in _cache:
        _cache["nc"] = _build(GPS_TREE)
    nc = _cache["nc"]

    sfT, logi16, maskT, order, bad = _host_prep(logits, targets)
    in_maps = []
    for ci in range(NCORES):
        sl = slice(ci * BC, (ci + 1) * BC)
        in_maps.append({
            "sft": sfT[ci],
            "logi": np.ascontiguousarray(logi16[sl]),
            "msk": maskT[ci],
        })
    res = bass_utils.run_bass_kernel_spmd(nc, in_maps, core_ids=list(range(NCORES)))
    outs = np.concatenate([r["outv"] for r in res.results], axis=0)  # [B, 3] permuted
    a31 = outs[:, 0].astype(np.float64)
    a32 = outs[:, 1].astype(np.float64)
    lz = outs[:, 2].astype(np.float64)
    ll_p = np.log(a31 + a32) - lz
    ll = np.empty(B, np.float64)
    ll[order] = ll_p
    if bad:
        bi = np.array(bad, np.int64)
        ll[bi] = _ll_exact(logits[bi], targets.reshape(B, L)[bi])
    loss = np.mean(-ll / L)
    return np.float32(loss)
